# revision 15
# baseline (speedup 1.0000x reference)
"""Multi-head attention on 8 Trainium2 NeuronCores (v2, fp16 + flipped attnV).

Sharding: data-parallel over batch (4) x tensor-parallel over head-groups (2).
Core c handles batch c//2, heads [8*(c%2), 8*(c%2)+8). Each core computes its
partial out-projection (over its 512 channels); host sums the pair per batch.

Device-side design (per core, all values fp16; PSUM accumulation f32):
  Q^T, K^T  [512, 2048] channel-major (W chunks stationary)
  V         [2048, 8*65] token-major (65 cols/head: 64 V + ones column)
  scores^T[s,t] in PSUM chunks A [128,2048] / B [128,1024] (pattern 4,2,4,2,4)
  exp on ACT (scale=1/8) -> P fp16 in SBUF; mask-mul on DVE (fp16 2x mode)
  attnV flipped: lhsT = P^T chunk [128s,128t], rhs = V[s,65] -> acc[t, 4*65]
  normalize: DVE recip of denom col + Pool tensor_scalar -> attn_n [t, 4*64]
  PE transpose (identity) -> tT psum fp16 [d, t] -> Pool copy -> onrm
  out-proj: lhsT = Wo^T chunks, rhs = onrm -> op psum -> Pool copy -> DMA out
"""
import sys

sys.path.insert(0, "/opt/trn_rl_repo")

import numpy as np

import concourse.bass as bass
import concourse.mybir as mybir
import concourse.tile as tile
from concourse import bacc
from concourse.bass_utils import run_bass_kernel_spmd

D_MODEL = 1024
NUM_HEADS = 16
DK = 64
B, S = 4, 2048
NCORES = 8
OG = 512            # channels per head-group
HPG = 8             # heads per group
IC = D_MODEL // 128  # 8 contraction chunks
F32 = mybir.dt.float32
F16 = mybir.dt.float16

# scores: 8 chunks of 2 s-chunks each, alternating psum rings A0/A1.
# PSUM banks: A0=2, A1=2, acc=1, tT=1, op=1, op2=1  (8 total)
NCHUNK = 8


def build_module():
    nc = bacc.Bacc("TRN2", target_bir_lowering=False, debug=False,
                   num_devices=NCORES)
    XQT = nc.dram_tensor("XQT", [D_MODEL, S], F16, kind="ExternalInput").ap()
    XKT = nc.dram_tensor("XKT", [D_MODEL, S], F16, kind="ExternalInput").ap()
    XVT = nc.dram_tensor("XVT", [D_MODEL, S], F16, kind="ExternalInput").ap()
    WQT = nc.dram_tensor("WQT", [D_MODEL, OG], F16, kind="ExternalInput").ap()
    WKT = nc.dram_tensor("WKT", [D_MODEL, OG], F16, kind="ExternalInput").ap()
    WVT = nc.dram_tensor("WVT", [D_MODEL, OG], F16, kind="ExternalInput").ap()
    WOT = nc.dram_tensor("WOT", [OG, D_MODEL], F16, kind="ExternalInput").ap()
    MASKT = nc.dram_tensor("MASKT", [S, S], F16, kind="ExternalInput").ap()
    BQ = nc.dram_tensor("BQ", [OG], F32, kind="ExternalInput").ap()
    BK = nc.dram_tensor("BK", [OG], F32, kind="ExternalInput").ap()
    BV = nc.dram_tensor("BV", [1, OG], F16, kind="ExternalInput").ap()
    IDT = nc.dram_tensor("IDT", [128, 128], F16, kind="ExternalInput").ap()
    OUTT = nc.dram_tensor("OUTT", [D_MODEL, S], F32, kind="ExternalOutput").ap()

    Exp = mybir.ActivationFunctionType.Exp

    with tile.TileContext(nc) as tc:
        with tc.tile_pool(name="persist", bufs=1) as pp, \
             tc.tile_pool(name="qkpool", bufs=1) as qkp:
            # projection outputs, resident through the whole kernel
            qt = [qkp.tile([128, S], F16, name=f"qt{j}") for j in range(4)]
            kt = [qkp.tile([128, S], F16, name=f"kt{j}") for j in range(4)]
            bq_t = pp.tile([128, 4], F32, name="bq_t")
            bk_t = pp.tile([128, 4], F32, name="bk_t")
            bv_t = pp.tile([1, OG], F16, name="bv_t")
            id_t = pp.tile([128, 128], F16, name="id_t")
            ones_t = pp.tile([1, 128], F16, name="ones_t")

            nc.sync.dma_start(out=bq_t[:], in_=BQ.rearrange("(j p) -> p j", p=128))
            nc.sync.dma_start(out=bk_t[:], in_=BK.rearrange("(j p) -> p j", p=128))
            nc.sync.dma_start(out=bv_t[:], in_=BV)
            nc.sync.dma_start(out=id_t[:], in_=IDT)
            nc.vector.memset(ones_t[:], 1.0)

            # ---------------- Phase A: projections ----------------
            vtp = tc.alloc_tile_pool(name="vtpool", bufs=1)
            vt = [vtp.tile([128, HPG * 65], F16, name=f"vt{j}")
                  for j in range(16)]
            with tc.tile_pool(name="wpool", bufs=1) as wp, \
                 tc.tile_pool(name="xpool", bufs=9) as xp, \
                 tc.tile_pool(name="psA", bufs=1, space="PSUM") as psA:
                wq = [wp.tile([128, OG], F16, name=f"wq{i}") for i in range(IC)]
                wk = [wp.tile([128, OG], F16, name=f"wk{i}") for i in range(IC)]
                wv = [wp.tile([128, OG], F16, tag=f"wq{i}", name=f"wv{i}")
                      for i in range(IC)]

                # K^T first, then V (aliases wk), then Q^T — so attention can
                # start as soon as Q's first blocks land.
                engs = (nc.sync, nc.scalar, nc.gpsimd)

                def qk_proj(xdram, wdram, wts, outts, bias):
                    xts = []
                    for i in range(IC):
                        xt = xp.tile([128, S], F16, tag="xt", name=f"xt{i}")
                        engs[i % 3].dma_start(
                            out=xt[:], in_=xdram[i * 128:(i + 1) * 128, :])
                        engs[(i + 1) % 3].dma_start(
                            out=wts[i][:], in_=wdram[i * 128:(i + 1) * 128, :])
                        xts.append(xt)
                    for rnd in range(2):
                        tcs = (2 * rnd, 2 * rnd + 1)
                        psums = {}
                        for och in range(4):
                            for ti, t in enumerate(tcs):
                                psums[(och, t)] = psA.tile(
                                    [128, 512], F32, tag=f"pj{och}_{ti}",
                                    name=f"pj{och}_{ti}")
                        for i in range(IC):
                            for och in range(4):
                                for t in tcs:
                                    nc.tensor.matmul(
                                        psums[(och, t)],
                                        wts[i][:, och * 128:(och + 1) * 128],
                                        xts[i][:, t * 512:(t + 1) * 512],
                                        start=(i == 0), stop=(i == IC - 1))
                        for och in range(4):
                            for t in tcs:
                                nc.scalar.activation(
                                    outts[och][:, t * 512:(t + 1) * 512],
                                    psums[(och, t)],
                                    mybir.ActivationFunctionType.Identity,
                                    bias=bias[:, och:och + 1])

                qk_proj(XKT, WKT, wk, kt, bk_t)

                # -------- V projection (wv aliases wk; K^T is done) ----
                for tch in range(16):
                    ocol = vt[tch][:].rearrange("p (h e) -> p h e", h=HPG)[:, :, 64:65]
                    nc.vector.memset(ocol, 1.0)
                xts = []
                for i in range(IC):
                    xt = xp.tile([128, S], F16, tag="xt", name=f"xv{i}")
                    engs[i % 3].dma_start(
                        out=xt[:], in_=XVT[i * 128:(i + 1) * 128, :])
                    engs[(i + 1) % 3].dma_start(
                        out=wv[i][:], in_=WVT[i * 128:(i + 1) * 128, :])
                    xts.append(xt)
                for vrnd in range(2):
                    tchs = list(range(8 * vrnd, 8 * vrnd + 8))
                    psums = {tch: psA.tile(
                        [128, 512], F32, tag=f"pj{(tch % 8) // 2}_{tch % 2}",
                        name=f"pv{tch % 8}") for tch in tchs}
                    for i in range(IC):
                        for tch in tchs:
                            nc.tensor.matmul(
                                psums[tch], xts[i][:, tch * 128:(tch + 1) * 128],
                                wv[i][:], start=(i == 0), stop=False)
                    for tch in tchs:
                        # bias row: ones[t] (x) bv[o], closes the accum group
                        nc.tensor.matmul(
                            psums[tch], ones_t[0:1, 0:128], bv_t[0:1, :],
                            start=False, stop=True)
                        nc.vector.tensor_copy(
                            vt[tch][:].rearrange(
                                "p (h e) -> p h e", h=HPG)[:, :, 0:64],
                            psums[tch][:].rearrange("p (h d) -> p h d", h=HPG))

                qk_proj(XQT, WQT, wq, qt, bq_t)

            # ---------------- Phase B/C: attention + out-proj ----------------
            with tc.tile_pool(name="wopool", bufs=1) as wop, \
                 tc.tile_pool(name="mpool", bufs=2) as mp, \
                 tc.tile_pool(name="ptpool", bufs=2) as ptp, \
                 tc.tile_pool(name="wkpool", bufs=2) as wkp, \
                 tc.tile_pool(name="onrmpool", bufs=2) as onp, \
                 tc.tile_pool(name="stgpool", bufs=2) as sgp, \
                 tc.tile_pool(name="psB", bufs=1, space="PSUM") as psB:
                wo = [wop.tile([128, D_MODEL], F16, name=f"wo{j}")
                      for j in range(4)]
                for j in range(4):
                    nc.sync.dma_start(out=wo[j][:],
                                      in_=WOT[j * 128:(j + 1) * 128, :])
                def load_mask(t):
                    mh = mp.tile([128, 16 * 512], F16, tag="mask", name="mask_t")
                    for s8 in range(16):
                        engs[s8 % 3].dma_start(
                            out=mh[:, s8 * 512:(s8 + 1) * 512],
                            in_=MASKT[s8 * 128:(s8 + 1) * 128,
                                      t * 512:(t + 1) * 512])
                    return mh

                # software-pipelined: head state flows one step behind
                tT_holder = [None]

                def attn_tail(state, onrm):
                    """attnV + normalize + transpose for a finished head."""
                    h, pts = state
                    acc = psB.tile([128, 4 * 65], F32, tag="acc", name="acc")
                    if h % 2 == 0:
                        tT_holder[0] = psB.tile([128, 512], F16, tag="tT",
                                                name="tT")
                    tT = tT_holder[0]
                    # u-major accumulation (interleaved starts in one psum
                    # tile mis-accumulate on HW)
                    for u in range(4):
                        for sc in range(16):
                            pt = pts[sc // 4]
                            off = (sc % 4) * 512 + u * 128
                            nc.tensor.matmul(
                                acc[:, u * 65:(u + 1) * 65],
                                pt[:, off:off + 128],
                                vt[sc][:, h * 65:(h + 1) * 65],
                                start=(sc == 0), stop=(sc == 15),
                                skip_group_check=True)
                    ho = (h % 2) * 64
                    rc = wkp.tile([128, 4], F32, tag="rc", name="rc")
                    att = wkp.tile([128, 256], F16, tag="att", name="att")
                    nc.vector.reciprocal(
                        rc[:],
                        acc[:].rearrange("p (q e) -> p q e", q=4)[:, :, 64])
                    for u in range(4):
                        nc.vector.tensor_scalar_mul(
                            att[:, u * 64:(u + 1) * 64],
                            acc[:, u * 65:u * 65 + 64], rc[:, u:u + 1])
                    for u in range(4):
                        nc.tensor.transpose(
                            tT[ho:ho + 64, u * 128:(u + 1) * 128],
                            att[:, u * 64:(u + 1) * 64], id_t[:])
                    if h % 2 == 1:
                        nc.vector.tensor_copy(onrm[h // 2][:], tT[:])

                def out_proj(t, onrm, och):
                    opt = psB.tile([128, 512], F32,
                                   tag="op" if och % 2 == 0 else "op2",
                                   name="op")
                    for cch in range(4):
                        nc.tensor.matmul(
                            opt[:],
                            wo[cch][:, och * 128:(och + 1) * 128],
                            onrm[cch][:], start=(cch == 0), stop=(cch == 3))
                    stg = sgp.tile([128, 512], F32, tag="stg", name="stg")
                    nc.vector.tensor_copy(stg[:], opt[:])
                    nc.gpsimd.dma_start(
                        out=OUTT[och * 128:(och + 1) * 128,
                                 t * 512:(t + 1) * 512],
                        in_=stg[:])

                mh = load_mask(0)
                state = None        # finished head awaiting attnV tail
                pending_op = None   # (t, onrm) awaiting out-projection
                onrm = None
                for t in range(4):
                    mh_next = None
                    onrm_prev, onrm = onrm, [
                        onp.tile([128, 512], F16, tag=f"onrm{j}",
                                 name=f"onrm{j}") for j in range(4)]
                    for h in range(HPG):
                        ht, ho = h // 2, (h % 2) * 64
                        pts = []
                        for ci in range(NCHUNK):
                            ps = psB.tile([128, 1024], F32, tag=f"A{ci % 2}",
                                          name=f"A{ci % 2}")
                            if ci % 2 == 0:
                                pt = ptp.tile([128, 2048], F16,
                                              tag=f"p{ci // 2}",
                                              name=f"p{ci // 2}")
                                pts.append(pt)
                            else:
                                pt = pts[-1]
                            for i in range(2):
                                sc = 2 * ci + i
                                nc.tensor.matmul(
                                    ps[:, i * 512:(i + 1) * 512],
                                    kt[ht][ho:ho + 64, sc * 128:(sc + 1) * 128],
                                    qt[ht][ho:ho + 64, t * 512:(t + 1) * 512],
                                    start=True, stop=True)
                            if ci == 0:
                                # previous head's tail runs on PE while this
                                # head's first scores chunk exps/masks
                                if state is not None:
                                    attn_tail(state, onrm if state[0] != 7
                                              else onrm_prev)
                                    state = None
                                # then one deferred out-projection column
                                if pending_op is not None:
                                    out_proj(pending_op[0], pending_op[1], h)
                                    if h == 7:
                                        pending_op = None
                            nc.scalar.activation(
                                pt[:, (ci % 2) * 1024:(ci % 2 + 1) * 1024],
                                ps, Exp, scale=0.125)
                            if ci % 2 == 1:
                                # mask multiply over the 2048-wide pair
                                pair = ci // 2
                                meng = nc.gpsimd if (
                                    pair == 1 or (pair == 3 and h % 4 == 0)
                                ) else nc.vector
                                meng.tensor_mul(
                                    pt[:], pt[:],
                                    mh[:, pair * 2048:(pair + 1) * 2048])
                        state = (h, pts)
                        if h == 4 and t < 3:
                            mh_next = load_mask(t + 1)
                    if t < 3:
                        pending_op = (t, onrm)
                        mh = mh_next
                # drain: last head tail + last t's out-projections
                attn_tail(state, onrm)
                for och in range(8):
                    out_proj(3, onrm, och)

            vtp.release()

    nc.compile()
    return nc


_NC_CACHE = {}


def _get_module():
    if "nc" not in _NC_CACHE:
        _NC_CACHE["nc"] = build_module()
    return _NC_CACHE["nc"]


def kernel(q, k, v, mask, Wq, bq, Wk, bk, Wv, bv, Wo, bo, **_ignored):
    q = np.asarray(q, dtype=np.float32)
    k = np.asarray(k, dtype=np.float32)
    v = np.asarray(v, dtype=np.float32)
    mask = np.asarray(mask)
    Wq, Wk, Wv, Wo = (np.asarray(w, dtype=np.float32) for w in (Wq, Wk, Wv, Wo))
    bq, bk, bv, bo = (np.asarray(b_, dtype=np.float32) for b_ in (bq, bk, bv, bo))

    maskT = (np.ascontiguousarray(mask[0, 0].T) != 0).astype(np.float16)
    idm = np.eye(128, dtype=np.float16)

    xT = {}
    for b_ in range(B):
        xT[("q", b_)] = np.ascontiguousarray(q[b_].T).astype(np.float16)
        xT[("k", b_)] = np.ascontiguousarray(k[b_].T).astype(np.float16)
        xT[("v", b_)] = np.ascontiguousarray(v[b_].T).astype(np.float16)
    wslice = {}
    for hg in range(2):
        og = hg * OG
        wslice[("q", hg)] = np.ascontiguousarray(Wq[og:og + OG, :].T).astype(np.float16)
        wslice[("k", hg)] = np.ascontiguousarray(Wk[og:og + OG, :].T).astype(np.float16)
        wslice[("v", hg)] = np.ascontiguousarray(Wv[og:og + OG, :].T).astype(np.float16)
        wslice[("o", hg)] = np.ascontiguousarray(Wo[:, og:og + OG].T).astype(np.float16)

    in_maps = []
    for c in range(NCORES):
        b_, hg = c // 2, c % 2
        og = hg * OG
        in_maps.append({
            "XQT": xT[("q", b_)], "XKT": xT[("k", b_)], "XVT": xT[("v", b_)],
            "WQT": wslice[("q", hg)], "WKT": wslice[("k", hg)],
            "WVT": wslice[("v", hg)], "WOT": wslice[("o", hg)],
            "MASKT": maskT,
            "BQ": bq[og:og + OG].astype(np.float32),
            "BK": bk[og:og + OG].astype(np.float32),
            "BV": bv[og:og + OG].reshape(1, OG).astype(np.float16),
            "IDT": idm,
        })

    nc = _get_module()
    res = run_bass_kernel_spmd(nc, in_maps, list(range(NCORES)))

    out = np.empty((B, S, D_MODEL), np.float32)
    for b_ in range(B):
        acc = res.results[2 * b_]["OUTT"] + res.results[2 * b_ + 1]["OUTT"]
        out[b_] = acc.T + bo
    return out
